# revision 10
# baseline (speedup 1.0000x reference)
"""Multi-head self-attention with RoPE on 8 Trainium2 NeuronCores.

Problem: B=2, S=2048, D_MODEL=2048, 16 heads x d_k=128, causal, RoPE on Q/K.

Sharding (hardcoded): core c -> batch b=c//4, head group g=c%4 (heads 4g..4g+3).
Data parallel on batch, tensor parallel on heads; q/k/v projections column-
sharded, output projection row-sharded with the partial sums reduced on host.

Device kernel (identical program on all 8 cores, different data). All matmul
operands are bf16 (PSUM accumulation stays fp32; end-to-end rel err ~5e-3,
well inside the 2e-2 gate), which halves DMA traffic vs fp32 and enables the
PE fast-weight-load path that fp32r weights cannot use. Layouts avoid all
on-chip transposes:
    QT/KT: (d_k, S) per head  - projection with weight-subtile stationary
    V:     (S, d_k*4) natural - projection with x-subtile stationary
    S^T = KT_tile.T @ QT -> (k, q); softmax sums over partitions via a
    ones-vector matmul; AV: V_tile.T @ P~T -> (d, q); output projection
    woT_tile.T @ OT -> partial out^T (e, q).

Schedule highlights:
  - x^T is loaded into SBUF once (bf16, 8.4MB) and stays resident; the
    V-pass runs et-outer over chunk pairs so compute starts ~2us into the
    x DMA stream instead of waiting for all of it.
  - Attention for chunk qc is emitted after the projection of chunk qc+1,
    so the RoPE drain (DVE) of a chunk overlaps the next chunk's
    projection matmuls and the PE never waits on the vector engine.
  - Causal diagonal blocks are trimmed to their live column range
    (512/384/256/128 wide); only the leading 128 columns of each are
    triangular, masked with one shared (128,128) lower-triangle multiply.
  - softmax normalization uses reciprocal_approx_fast (the exact DVE
    reciprocal costs 3.3us per call on a (1,512) operand).
  - RoPE: even/odd d_k interleave is pre-permuted into the wq/wk rows on
    host; rows 0:64 = even dims, 64:128 = odd dims per head.
"""

import sys

sys.path.insert(0, "/opt/trn_rl_repo")

import math
from contextlib import ExitStack

import ml_dtypes
import numpy as np

import concourse.bass as bass
import concourse.mybir as mybir
import concourse.tile as tile
from concourse import bacc
from concourse.bass_utils import run_bass_kernel_spmd

f32 = mybir.dt.float32
bf16 = mybir.dt.bfloat16

B = 2
S = 2048
D = 2048
H = 16
DK = 128
H_CORE = 4  # heads per core
DL = H_CORE * DK  # local feature dim 512
ET = D // 128  # 16 e-tiles (contraction over d_model)
QC = S // 512  # 4 q-chunks
THETA = 10000.0
SCALE = 1.0 / math.sqrt(DK)

N_CORES = 8


def _build():
    nc = bacc.Bacc("TRN2", target_bir_lowering=False, debug=False)

    xT_d = nc.dram_tensor("xT", [D, S], bf16, kind="ExternalInput")
    wqT_d = nc.dram_tensor("wqT", [D, DL], bf16, kind="ExternalInput")
    wkT_d = nc.dram_tensor("wkT", [D, DL], bf16, kind="ExternalInput")
    wvT_d = nc.dram_tensor("wvT", [D, DL], bf16, kind="ExternalInput")
    woT_d = nc.dram_tensor("woT", [DL, D], bf16, kind="ExternalInput")
    cosT_d = nc.dram_tensor("cosT", [64, S], bf16, kind="ExternalInput")
    sinT_d = nc.dram_tensor("sinT", [64, S], bf16, kind="ExternalInput")
    tri_d = nc.dram_tensor("tri", [128, 512], bf16, kind="ExternalInput")
    outT_d = nc.dram_tensor("outT", [D, S], bf16, kind="ExternalOutput")

    Exp = mybir.ActivationFunctionType.Exp

    with tile.TileContext(nc) as tc:
      with tc.tile_pool(name="const", bufs=1) as const, \
           tc.tile_pool(name="xpool", bufs=1) as xpool, \
           tc.tile_pool(name="persist", bufs=1) as persist, \
           tc.tile_pool(name="wqp", bufs=24) as wqp, \
           tc.tile_pool(name="wkp", bufs=24) as wkp, \
           tc.tile_pool(name="ropet", bufs=1) as ropet, \
           tc.tile_pool(name="psum", bufs=1, space="PSUM") as psum:

        xs = [xpool.tile([128, S], bf16, tag=f"x{et}", name=f"x{et}")
              for et in range(ET)]
        V = [persist.tile([128, DL], bf16, tag=f"v{st}", name=f"v{st}")
             for st in range(ET)]
        OT = [persist.tile([DK, S], bf16, tag=f"ot{h}", name=f"ot{h}")
              for h in range(H_CORE)]

        def load_wqk(p):
            wq_sb, wk_sb = [], []
            for et in range(ET):
                wqt = wqp.tile([128, 256], bf16, tag="wq", name=f"wq{p}")
                nc.sync.dma_start(
                    wqt[:],
                    wqT_d[et * 128 : (et + 1) * 128, p * 256 : (p + 1) * 256],
                )
                wq_sb.append(wqt)
                wkt = wkp.tile([128, 256], bf16, tag="wk", name=f"wk{p}")
                nc.sync.dma_start(
                    wkt[:],
                    wkT_d[et * 128 : (et + 1) * 128, p * 256 : (p + 1) * 256],
                )
                wk_sb.append(wkt)
            return wq_sb, wk_sb

        # ---- V pass: et-outer over chunk pairs so the PE starts after the
        # first x/wv tiles land rather than after the full 10.5MB stream.
        with tc.tile_pool(name="wvp", bufs=16) as wvp:
            wv_sb = []
            for et in range(ET):
                wt = wvp.tile([128, DL], bf16, tag="wv", name="wv")
                nc.sync.dma_start(wt[:], wvT_d[et * 128 : (et + 1) * 128, :])
                wv_sb.append(wt)
                nc.sync.dma_start(
                    xs[et][:, 0:1024], xT_d[et * 128 : (et + 1) * 128, 0:1024]
                )
            for et in range(ET):
                nc.sync.dma_start(
                    xs[et][:, 1024:2048],
                    xT_d[et * 128 : (et + 1) * 128, 1024:2048],
                )
            for qcp in range(2):  # chunk pairs (0,1) then (2,3)
                vacc = [
                    psum.tile([128, DL], f32, tag=f"t{i}", name=f"vacc{i}")
                    for i in range(8)
                ]
                for et in range(ET):
                    for half in range(2):
                        qc = 2 * qcp + half
                        for sl in range(4):
                            nc.tensor.matmul(
                                vacc[4 * half + sl][:],
                                xs[et][:, qc * 512 + sl * 128
                                       : qc * 512 + (sl + 1) * 128],
                                wv_sb[et][:],
                                start=(et == 0),
                                stop=(et == ET - 1),
                            )
                for half in range(2):
                    for sl in range(4):
                        nc.scalar.copy(
                            V[(2 * qcp + half) * 4 + sl][:],
                            vacc[4 * half + sl][:],
                        )
            wqk0 = load_wqk(0)

        # ---- constants + prefetches (land during the V pass) ------------
        cos2 = const.tile([128, S], bf16, tag="cos2")
        sin2 = const.tile([128, S], bf16, tag="sin2")
        nc.sync.dma_start(cos2[0:64, :], cosT_d[:, :])
        nc.sync.dma_start(cos2[64:128, :], cosT_d[:, :])
        nc.sync.dma_start(sin2[0:64, :], sinT_d[:, :])
        nc.sync.dma_start(sin2[64:128, :], sinT_d[:, :])
        mz = const.tile([128, 512], bf16, tag="mz")
        nc.sync.dma_start(mz[:], tri_d[:, :])
        ones_f = const.tile([128, 1], f32, tag="ones_f")
        nc.vector.memset(ones_f[:], 1.0)
        ones = const.tile([128, 1], bf16, tag="ones")
        nc.vector.tensor_copy(ones[:], ones_f[:])
        wqk1 = load_wqk(1)
        with tc.tile_pool(name="wop", bufs=64) as wop, \
             tc.tile_pool(name="stg", bufs=4) as stg:
            wo_t = []
            for et in range(ET):
                for hh in range(H_CORE):
                    wt = wop.tile([128, 128], bf16, tag="wo", name="wo")
                    nc.sync.dma_start(
                        wt[:],
                        woT_d[hh * 128 : (hh + 1) * 128,
                              et * 128 : (et + 1) * 128],
                    )
                    wo_t.append(wt)

            def rope(dst, ev, od, qs):
                """ev/od: PSUM accumulators (128,512), rows [hA;hB]."""
                c = cos2[:, qs]
                sn = sin2[:, qs]
                m1 = ropet.tile([128, 512], bf16, tag="m1")
                m2 = ropet.tile([128, 512], bf16, tag="m2")
                n1 = ropet.tile([128, 512], bf16, tag="n1")
                n2 = ropet.tile([128, 512], bf16, tag="n2")
                nc.vector.tensor_mul(m1[:], ev[:], c)
                nc.vector.tensor_mul(m2[:], od[:], sn)
                nc.vector.tensor_mul(n1[:], ev[:], sn)
                nc.vector.tensor_mul(n2[:], od[:], c)
                nc.vector.tensor_sub(dst[0][0:64, qs], m1[0:64, :],
                                     m2[0:64, :])
                nc.vector.tensor_sub(dst[1][0:64, qs], m1[64:128, :],
                                     m2[64:128, :])
                nc.vector.tensor_add(dst[0][64:128, qs], n1[0:64, :],
                                     n2[0:64, :])
                nc.vector.tensor_add(dst[1][64:128, qs], n1[64:128, :],
                                     n2[64:128, :])

            # ---- projection / rope / attention pipeline ----------------
            # Emission (= per-engine priority) order lags each chunk's
            # attention one slot behind its projection:
            #   proj(0) rope(0) proj(1) attn(0) rope(1) proj(2) attn(1) ...
            # so rope(qc) on DVE overlaps proj(qc+1) on the PE, and the
            # proj accumulators (t0-t3) are freed by rope's 4 leading muls.
            with tc.tile_pool(name="qk", bufs=2) as qkp, \
                 tc.tile_pool(name="ptp", bufs=6) as ptp, \
                 tc.tile_pool(name="paccp", bufs=2) as paccp, \
                 tc.tile_pool(name="smallp", bufs=2) as smallp:

                QTs, KTs = {}, {}

                def proj(p, qc, wq_sb, wk_sb):
                    if qc == 0:
                        QTs[p] = [qkp.tile([DK, S], bf16, tag=f"qt{i}",
                                           name=f"qt{p}_{i}")
                                  for i in range(2)]
                        KTs[p] = [qkp.tile([DK, S], bf16, tag=f"kt{i}",
                                           name=f"kt{p}_{i}")
                                  for i in range(2)]
                    qs = slice(qc * 512, (qc + 1) * 512)
                    qe = psum.tile([128, 512], f32, tag="t0", name="qe")
                    qo = psum.tile([128, 512], f32, tag="t1", name="qo")
                    ke = psum.tile([128, 512], f32, tag="t2", name="ke")
                    ko = psum.tile([128, 512], f32, tag="t3", name="ko")
                    for et in range(ET):
                        xt = xs[et]
                        nc.tensor.matmul(
                            qe[:], wq_sb[et][:, 0:128], xt[:, qs],
                            start=(et == 0), stop=(et == ET - 1),
                        )
                        nc.tensor.matmul(
                            qo[:], wq_sb[et][:, 128:256], xt[:, qs],
                            start=(et == 0), stop=(et == ET - 1),
                        )
                        nc.tensor.matmul(
                            ke[:], wk_sb[et][:, 0:128], xt[:, qs],
                            start=(et == 0), stop=(et == ET - 1),
                        )
                        nc.tensor.matmul(
                            ko[:], wk_sb[et][:, 128:256], xt[:, qs],
                            start=(et == 0), stop=(et == ET - 1),
                        )
                    rope(QTs[p], qe, qo, qs)
                    rope(KTs[p], ke, ko, qs)

                def attn(p, qc):
                    QT, KT = QTs[p], KTs[p]
                    ha = 2 * p
                    for hi, h in enumerate((ha, ha + 1)):
                        nkt = 4 * qc + 4
                        nquad = nkt // 4
                        LAG = 2  # scores run LAG blocks ahead of AV
                        den = psum.tile([1, 512], f32, tag="t6", name="den")
                        oacc = psum.tile([128, 512], f32, tag="t7",
                                         name="oacc")
                        pts = {}
                        pacc = [None] * nquad

                        def consume(kt):
                            j = kt - 4 * qc
                            o = 128 * j if j >= 0 else 0
                            cs = slice(o, 512)
                            pt = pts.pop(kt)
                            nc.tensor.matmul(
                                oacc[:, cs],
                                V[kt][:, h * 128 : (h + 1) * 128],
                                pt[:, cs],
                                start=(kt == 0),
                                stop=(kt == nkt - 1),
                                skip_group_check=True,
                            )

                        for kt in range(nkt):
                            j = kt - 4 * qc
                            o = 128 * j if j >= 0 else 0
                            cs = slice(o, 512)
                            sps = psum.tile(
                                [128, 512], f32,
                                tag=("t4" if kt % 2 == 0 else "t5"),
                                name="sps",
                            )
                            nc.tensor.matmul(
                                sps[:, cs],
                                KT[hi][:, kt * 128 : (kt + 1) * 128],
                                QT[hi][:, qc * 512 + o : (qc + 1) * 512],
                                start=True,
                                stop=True,
                            )
                            pt = ptp.tile([128, 512], bf16, tag="pt")
                            nc.scalar.activation(
                                pt[:, cs], sps[:, cs], Exp, scale=SCALE
                            )
                            if j >= 0:
                                # only cols [o, o+128) of a diagonal block
                                # are triangular; the rest is causal-valid
                                nc.vector.tensor_mul(
                                    pt[:, o : o + 128],
                                    pt[:, o : o + 128],
                                    mz[:, 384:512],
                                )
                            pts[kt] = pt
                            # den pre-accumulation: 3 DVE adds per quad of
                            # k-tiles replace 3 of 4 PE den streams. The
                            # diagonal quad's tiles are only valid on
                            # [128*j, 512), so its adds are range-limited
                            # (cols below that were never written).
                            m, r = divmod(kt, 4)
                            diag = m == qc
                            if r == 1:
                                pa = paccp.tile([128, 512], bf16,
                                                tag="pacc")
                                if diag:
                                    nc.gpsimd.tensor_copy(
                                        pa[:, 0:128], pts[kt - 1][:, 0:128]
                                    )
                                    nc.gpsimd.tensor_add(
                                        pa[:, 128:512],
                                        pts[kt - 1][:, 128:512],
                                        pt[:, 128:512],
                                    )
                                else:
                                    nc.gpsimd.tensor_add(
                                        pa[:], pts[kt - 1][:], pt[:]
                                    )
                                pacc[m] = pa
                            elif r > 1:
                                cs2 = slice(o, 512) if diag else slice(
                                    0, 512
                                )
                                nc.gpsimd.tensor_add(
                                    pacc[m][:, cs2],
                                    pacc[m][:, cs2],
                                    pt[:, cs2],
                                )
                                if r == 3:
                                    nc.tensor.matmul(
                                        den[:], ones[:], pacc[m][:],
                                        start=(m == 0),
                                        stop=(m == nquad - 1),
                                        skip_group_check=True,
                                    )
                            if kt >= LAG:
                                consume(kt - LAG)
                        for kt in range(max(0, nkt - LAG), nkt):
                            consume(kt)
                        qs = slice(qc * 512, (qc + 1) * 512)
                        rec = smallp.tile([1, 512], f32, tag="rec")
                        nc.vector.reciprocal_approx_fast(rec[:], den[:])
                        bc = smallp.tile([128, 512], f32, tag="bc")
                        nc.gpsimd.partition_broadcast(bc[:], rec[:])
                        nc.vector.tensor_mul(OT[h][:, qs], oacc[:], bc[:])

                # software pipeline over (pair, chunk)
                slots = [(p, qc) for p in range(2) for qc in range(QC)]
                wqks = {0: wqk0, 1: wqk1}
                prev = None
                for p, qc in slots:
                    proj(p, qc, *wqks[p])
                    if prev is not None:
                        attn(*prev)
                    prev = (p, qc)
                attn(*prev)

            # ---- output projection tail (weights already resident) ------
            for et in range(ET):
                for qc in range(QC):
                    qs = slice(qc * 512, (qc + 1) * 512)
                    facc = psum.tile(
                        [128, 512], f32, tag=f"t{(et * QC + qc) % 4}",
                        name="facc",
                    )
                    for hh in range(H_CORE):
                        nc.tensor.matmul(
                            facc[:],
                            wo_t[et * H_CORE + hh][:],
                            OT[hh][:, qs],
                            start=(hh == 0),
                            stop=(hh == H_CORE - 1),
                        )
                    st = stg.tile([128, 512], bf16, tag="stg")
                    if (et * QC + qc) % 3 == 2:
                        nc.vector.tensor_copy(st[:], facc[:])
                    else:
                        nc.scalar.copy(st[:], facc[:])
                    nc.sync.dma_start(
                        outT_d[et * 128 : (et + 1) * 128, qs], st[:]
                    )

    return nc


_NC = None


def _get_nc():
    global _NC
    if _NC is None:
        _NC = _build()
        _NC.compile()
    return _NC


def _rope_perm_rows():
    """Row permutation applied to wq/wk for one core's 4 heads.

    Per head-pair p: [hA even dims, hB even dims, hA odd dims, hB odd dims]
    so the device sees even/odd deinterleaved, pair-stacked projections.
    Returns indices into the local (4*DK,) head-row block.
    """
    idx = []
    for p in range(2):
        ha, hb = 2 * p, 2 * p + 1
        idx.extend(ha * DK + np.arange(0, DK, 2))
        idx.extend(hb * DK + np.arange(0, DK, 2))
        idx.extend(ha * DK + np.arange(1, DK, 2))
        idx.extend(hb * DK + np.arange(1, DK, 2))
    return np.asarray(idx)


def _host_tables(positions):
    """cos/sin tables (64, S) float32, matching the fp32 reference math."""
    dim_idx = np.arange(0, DK, 2, dtype=np.float32)
    freqs = np.float32(THETA) ** (dim_idx / np.float32(DK))
    angles = positions.astype(np.float32)[:, None] / freqs[None, :]  # (S, 64)
    return (
        np.ascontiguousarray(np.cos(angles).T.astype(np.float32)),
        np.ascontiguousarray(np.sin(angles).T.astype(np.float32)),
    )


def _make_in_maps(inputs):
    x = np.asarray(inputs["x"], dtype=np.float32)
    wq = np.asarray(inputs["wq"], dtype=np.float32)
    wk = np.asarray(inputs["wk"], dtype=np.float32)
    wv = np.asarray(inputs["wv"], dtype=np.float32)
    wo = np.asarray(inputs["wo"], dtype=np.float32)
    token_positions = np.asarray(inputs["token_positions"])

    perm = _rope_perm_rows()
    p_idx = np.arange(128)[:, None]
    f_idx = np.arange(128)[None, :]
    mz = np.zeros((128, 512), dtype=np.float32)
    mz[:, 384:512] = p_idx <= f_idx
    mz = mz.astype(ml_dtypes.bfloat16)

    in_maps = []
    for c in range(N_CORES):
        b = c // 4
        g = c % 4
        rows = slice(g * DL, (g + 1) * DL)
        cosT, sinT = _host_tables(token_positions[b])
        in_maps.append(
            {
                "xT": np.ascontiguousarray(x[b].T).astype(ml_dtypes.bfloat16),
                "wqT": np.ascontiguousarray(wq[rows][perm].T).astype(
                    ml_dtypes.bfloat16
                ),
                "wkT": np.ascontiguousarray(wk[rows][perm].T).astype(
                    ml_dtypes.bfloat16
                ),
                "wvT": np.ascontiguousarray(wv[rows].T).astype(
                    ml_dtypes.bfloat16
                ),
                "woT": np.ascontiguousarray(wo[:, rows].T).astype(
                    ml_dtypes.bfloat16
                ),
                "cosT": cosT.astype(ml_dtypes.bfloat16),
                "sinT": sinT.astype(ml_dtypes.bfloat16),
                "tri": mz,
            }
        )
    return in_maps


def kernel(x, wq, wk, wv, wo, token_positions):
    nc = _get_nc()
    in_maps = _make_in_maps(
        {
            "x": x,
            "wq": wq,
            "wk": wk,
            "wv": wv,
            "wo": wo,
            "token_positions": token_positions,
        }
    )
    res = run_bass_kernel_spmd(nc, in_maps, list(range(N_CORES)))

    out = np.zeros((B, S, D), dtype=np.float32)
    for c in range(N_CORES):
        out[c // 4] += res.results[c]["outT"].astype(np.float32).T
    return out


# revision 11
# speedup vs baseline: 1.5449x; 1.5449x over previous
"""Multi-head self-attention with RoPE on 8 Trainium2 NeuronCores.

Problem: B=2, S=2048, D_MODEL=2048, 16 heads x d_k=128, causal, RoPE on Q/K.

Sharding (hardcoded): core c -> batch b=c//4, head group g=c%4 (heads 4g..4g+3).
Data parallel on batch, tensor parallel on heads; q/k/v projections column-
sharded, output projection row-sharded with the partial sums reduced on host.

Device kernel (identical program on all 8 cores, different data). All matmul
operands are bf16 (PSUM accumulation stays fp32; end-to-end rel err ~5e-3,
well inside the 2e-2 gate), which halves DMA traffic vs fp32 and enables the
PE fast-weight-load path that fp32r weights cannot use. Layouts avoid all
on-chip transposes:
    QT/KT: (d_k, S) per head  - projection with weight-subtile stationary
    V:     (S, d_k*4) natural - projection with x-subtile stationary
    S^T = KT_tile.T @ QT -> (k, q); softmax sums over partitions via a
    ones-vector matmul; AV: V_tile.T @ P~T -> (d, q); output projection
    woT_tile.T @ OT -> partial out^T (e, q).

Schedule highlights:
  - x^T is loaded into SBUF once (bf16, 8.4MB) and stays resident; the
    V-pass runs et-outer over chunk pairs so compute starts ~2us into the
    x DMA stream instead of waiting for all of it.
  - Attention for chunk qc is emitted after the projection of chunk qc+1,
    so the RoPE drain (DVE) of a chunk overlaps the next chunk's
    projection matmuls and the PE never waits on the vector engine.
  - Causal diagonal blocks are trimmed to their live column range
    (512/384/256/128 wide); only the leading 128 columns of each are
    triangular, masked with one shared (128,128) lower-triangle multiply.
  - softmax normalization uses reciprocal_approx_fast (the exact DVE
    reciprocal costs 3.3us per call on a (1,512) operand).
  - RoPE: even/odd d_k interleave is pre-permuted into the wq/wk rows on
    host; rows 0:64 = even dims, 64:128 = odd dims per head.
"""

import sys

sys.path.insert(0, "/opt/trn_rl_repo")

import math
from contextlib import ExitStack

import ml_dtypes
import numpy as np

import concourse.bass as bass
import concourse.mybir as mybir
import concourse.tile as tile
from concourse import bacc
from concourse.bass_utils import run_bass_kernel_spmd

f32 = mybir.dt.float32
bf16 = mybir.dt.bfloat16

B = 2
S = 2048
D = 2048
H = 16
DK = 128
H_CORE = 4  # heads per core
DL = H_CORE * DK  # local feature dim 512
ET = D // 128  # 16 e-tiles (contraction over d_model)
QC = S // 512  # 4 q-chunks
THETA = 10000.0
SCALE = 1.0 / math.sqrt(DK)

N_CORES = 8


def _build():
    nc = bacc.Bacc("TRN2", target_bir_lowering=False, debug=False)

    xT_d = nc.dram_tensor("xT", [D, S], bf16, kind="ExternalInput")
    wqT_d = nc.dram_tensor("wqT", [D, DL], bf16, kind="ExternalInput")
    wkT_d = nc.dram_tensor("wkT", [D, DL], bf16, kind="ExternalInput")
    wvT_d = nc.dram_tensor("wvT", [D, DL], bf16, kind="ExternalInput")
    woT_d = nc.dram_tensor("woT", [DL, D], bf16, kind="ExternalInput")
    cosT_d = nc.dram_tensor("cosT", [64, S], bf16, kind="ExternalInput")
    sinT_d = nc.dram_tensor("sinT", [64, S], bf16, kind="ExternalInput")
    tri_d = nc.dram_tensor("tri", [128, 512], bf16, kind="ExternalInput")
    outT_d = nc.dram_tensor("outT", [D, S], bf16, kind="ExternalOutput")

    Exp = mybir.ActivationFunctionType.Exp

    with tile.TileContext(nc) as tc:
      with tc.tile_pool(name="const", bufs=1) as const, \
           tc.tile_pool(name="xpool", bufs=1) as xpool, \
           tc.tile_pool(name="persist", bufs=1) as persist, \
           tc.tile_pool(name="wqp", bufs=24) as wqp, \
           tc.tile_pool(name="wkp", bufs=24) as wkp, \
           tc.tile_pool(name="ropet", bufs=1) as ropet, \
           tc.tile_pool(name="psum", bufs=1, space="PSUM") as psum:

        xs = [xpool.tile([128, S], bf16, tag=f"x{et}", name=f"x{et}")
              for et in range(ET)]
        V = [persist.tile([128, DL], bf16, tag=f"v{st}", name=f"v{st}")
             for st in range(ET)]
        OT = [persist.tile([DK, S], bf16, tag=f"ot{h}", name=f"ot{h}")
              for h in range(H_CORE)]

        def load_wqk(p):
            wq_sb, wk_sb = [], []
            for et in range(ET):
                wqt = wqp.tile([128, 256], bf16, tag="wq", name=f"wq{p}")
                nc.sync.dma_start(
                    wqt[:],
                    wqT_d[et * 128 : (et + 1) * 128, p * 256 : (p + 1) * 256],
                )
                wq_sb.append(wqt)
                wkt = wkp.tile([128, 256], bf16, tag="wk", name=f"wk{p}")
                nc.sync.dma_start(
                    wkt[:],
                    wkT_d[et * 128 : (et + 1) * 128, p * 256 : (p + 1) * 256],
                )
                wk_sb.append(wkt)
            return wq_sb, wk_sb

        # ---- V pass: et-outer over chunk pairs so the PE starts after the
        # first x/wv tiles land rather than after the full 10.5MB stream.
        with tc.tile_pool(name="wvp", bufs=16) as wvp:
            wv_sb = []
            for et in range(ET):
                wt = wvp.tile([128, DL], bf16, tag="wv", name="wv")
                nc.sync.dma_start(wt[:], wvT_d[et * 128 : (et + 1) * 128, :])
                wv_sb.append(wt)
                nc.sync.dma_start(
                    xs[et][:, 0:1024], xT_d[et * 128 : (et + 1) * 128, 0:1024]
                )
            for et in range(ET):
                nc.sync.dma_start(
                    xs[et][:, 1024:2048],
                    xT_d[et * 128 : (et + 1) * 128, 1024:2048],
                )
            for qcp in range(2):  # chunk pairs (0,1) then (2,3)
                vacc = [
                    psum.tile([128, DL], f32, tag=f"t{i}", name=f"vacc{i}")
                    for i in range(8)
                ]
                for et in range(ET):
                    for half in range(2):
                        qc = 2 * qcp + half
                        for sl in range(4):
                            nc.tensor.matmul(
                                vacc[4 * half + sl][:],
                                xs[et][:, qc * 512 + sl * 128
                                       : qc * 512 + (sl + 1) * 128],
                                wv_sb[et][:],
                                start=(et == 0),
                                stop=(et == ET - 1),
                            )
                for half in range(2):
                    for sl in range(4):
                        nc.scalar.copy(
                            V[(2 * qcp + half) * 4 + sl][:],
                            vacc[4 * half + sl][:],
                        )
            wqk0 = load_wqk(0)

        # ---- constants + prefetches (land during the V pass) ------------
        cos2 = const.tile([128, S], bf16, tag="cos2")
        sin2 = const.tile([128, S], bf16, tag="sin2")
        nc.sync.dma_start(cos2[0:64, :], cosT_d[:, :])
        nc.sync.dma_start(cos2[64:128, :], cosT_d[:, :])
        nc.sync.dma_start(sin2[0:64, :], sinT_d[:, :])
        nc.sync.dma_start(sin2[64:128, :], sinT_d[:, :])
        mz = const.tile([128, 512], bf16, tag="mz")
        nc.sync.dma_start(mz[:], tri_d[:, :])
        ones_f = const.tile([128, 1], f32, tag="ones_f")
        nc.vector.memset(ones_f[:], 1.0)
        ones = const.tile([128, 1], bf16, tag="ones")
        nc.vector.tensor_copy(ones[:], ones_f[:])
        wqk1 = load_wqk(1)
        with tc.tile_pool(name="wop", bufs=64) as wop, \
             tc.tile_pool(name="stg", bufs=4) as stg:
            wo_t = []
            for et in range(ET):
                for hh in range(H_CORE):
                    wt = wop.tile([128, 128], bf16, tag="wo", name="wo")
                    nc.sync.dma_start(
                        wt[:],
                        woT_d[hh * 128 : (hh + 1) * 128,
                              et * 128 : (et + 1) * 128],
                    )
                    wo_t.append(wt)

            def rope(dst, ev, od, qs):
                """ev/od: PSUM accumulators (128,512), rows [hA;hB]."""
                c = cos2[:, qs]
                sn = sin2[:, qs]
                m1 = ropet.tile([128, 512], bf16, tag="m1")
                m2 = ropet.tile([128, 512], bf16, tag="m2")
                n1 = ropet.tile([128, 512], bf16, tag="n1")
                n2 = ropet.tile([128, 512], bf16, tag="n2")
                nc.vector.tensor_mul(m1[:], ev[:], c)
                nc.vector.tensor_mul(m2[:], od[:], sn)
                nc.vector.tensor_mul(n1[:], ev[:], sn)
                nc.vector.tensor_mul(n2[:], od[:], c)
                nc.vector.tensor_sub(dst[0][0:64, qs], m1[0:64, :],
                                     m2[0:64, :])
                nc.vector.tensor_sub(dst[1][0:64, qs], m1[64:128, :],
                                     m2[64:128, :])
                nc.vector.tensor_add(dst[0][64:128, qs], n1[0:64, :],
                                     n2[0:64, :])
                nc.vector.tensor_add(dst[1][64:128, qs], n1[64:128, :],
                                     n2[64:128, :])

            # ---- projection / rope / attention pipeline ----------------
            # Emission (= per-engine priority) order lags each chunk's
            # attention one slot behind its projection:
            #   proj(0) rope(0) proj(1) attn(0) rope(1) proj(2) attn(1) ...
            # so rope(qc) on DVE overlaps proj(qc+1) on the PE, and the
            # proj accumulators (t0-t3) are freed by rope's 4 leading muls.
            with tc.tile_pool(name="qk", bufs=2) as qkp, \
                 tc.tile_pool(name="ptp", bufs=6) as ptp, \
                 tc.tile_pool(name="paccp", bufs=2) as paccp, \
                 tc.tile_pool(name="smallp", bufs=2) as smallp:

                QTs, KTs = {}, {}

                def proj(p, qc, wq_sb, wk_sb):
                    if qc == 0:
                        QTs[p] = [qkp.tile([DK, S], bf16, tag=f"qt{i}",
                                           name=f"qt{p}_{i}")
                                  for i in range(2)]
                        KTs[p] = [qkp.tile([DK, S], bf16, tag=f"kt{i}",
                                           name=f"kt{p}_{i}")
                                  for i in range(2)]
                    qs = slice(qc * 512, (qc + 1) * 512)
                    qe = psum.tile([128, 512], f32, tag="t0", name="qe")
                    qo = psum.tile([128, 512], f32, tag="t1", name="qo")
                    ke = psum.tile([128, 512], f32, tag="t2", name="ke")
                    ko = psum.tile([128, 512], f32, tag="t3", name="ko")
                    for et in range(ET):
                        xt = xs[et]
                        nc.tensor.matmul(
                            qe[:], wq_sb[et][:, 0:128], xt[:, qs],
                            start=(et == 0), stop=(et == ET - 1),
                        )
                        nc.tensor.matmul(
                            qo[:], wq_sb[et][:, 128:256], xt[:, qs],
                            start=(et == 0), stop=(et == ET - 1),
                        )
                        nc.tensor.matmul(
                            ke[:], wk_sb[et][:, 0:128], xt[:, qs],
                            start=(et == 0), stop=(et == ET - 1),
                        )
                        nc.tensor.matmul(
                            ko[:], wk_sb[et][:, 128:256], xt[:, qs],
                            start=(et == 0), stop=(et == ET - 1),
                        )
                    rope(QTs[p], qe, qo, qs)
                    rope(KTs[p], ke, ko, qs)

                def attn(p, qc):
                    QT, KT = QTs[p], KTs[p]
                    ha = 2 * p
                    for hi, h in enumerate((ha, ha + 1)):
                        nkt = 4 * qc + 4
                        nquad = nkt // 4
                        LAG = 2  # scores run LAG blocks ahead of AV
                        den = psum.tile([1, 512], f32, tag="t6", name="den")
                        oacc = psum.tile([128, 512], f32, tag="t7",
                                         name="oacc")
                        pts = {}
                        pacc = [None] * nquad

                        def consume(kt):
                            j = kt - 4 * qc
                            o = 128 * j if j >= 0 else 0
                            cs = slice(o, 512)
                            pt = pts.pop(kt)
                            nc.tensor.matmul(
                                oacc[:, cs],
                                V[kt][:, h * 128 : (h + 1) * 128],
                                pt[:, cs],
                                start=(kt == 0),
                                stop=(kt == nkt - 1),
                                skip_group_check=True,
                            )

                        for kt in range(nkt):
                            j = kt - 4 * qc
                            o = 128 * j if j >= 0 else 0
                            cs = slice(o, 512)
                            sps = psum.tile(
                                [128, 512], f32,
                                tag=("t4" if kt % 2 == 0 else "t5"),
                                name="sps",
                            )
                            nc.tensor.matmul(
                                sps[:, cs],
                                KT[hi][:, kt * 128 : (kt + 1) * 128],
                                QT[hi][:, qc * 512 + o : (qc + 1) * 512],
                                start=True,
                                stop=True,
                            )
                            pt = ptp.tile([128, 512], bf16, tag="pt")
                            nc.scalar.activation(
                                pt[:, cs], sps[:, cs], Exp, scale=SCALE
                            )
                            if j >= 0:
                                # only cols [o, o+128) of a diagonal block
                                # are triangular; the rest is causal-valid
                                nc.vector.tensor_mul(
                                    pt[:, o : o + 128],
                                    pt[:, o : o + 128],
                                    mz[:, 384:512],
                                )
                            pts[kt] = pt
                            # den pre-accumulation: 3 DVE adds per quad of
                            # k-tiles replace 3 of 4 PE den streams. The
                            # diagonal quad's tiles are only valid on
                            # [128*j, 512), so its adds are range-limited
                            # (cols below that were never written).
                            m, r = divmod(kt, 4)
                            diag = m == qc
                            if r == 1:
                                pa = paccp.tile([128, 512], bf16,
                                                tag="pacc")
                                if diag:
                                    nc.vector.tensor_copy(
                                        pa[:, 0:128], pts[kt - 1][:, 0:128]
                                    )
                                    nc.vector.tensor_add(
                                        pa[:, 128:512],
                                        pts[kt - 1][:, 128:512],
                                        pt[:, 128:512],
                                    )
                                else:
                                    nc.vector.tensor_add(
                                        pa[:], pts[kt - 1][:], pt[:]
                                    )
                                pacc[m] = pa
                            elif r > 1:
                                cs2 = slice(o, 512) if diag else slice(
                                    0, 512
                                )
                                nc.vector.tensor_add(
                                    pacc[m][:, cs2],
                                    pacc[m][:, cs2],
                                    pt[:, cs2],
                                )
                                if r == 3:
                                    nc.tensor.matmul(
                                        den[:], ones[:], pacc[m][:],
                                        start=(m == 0),
                                        stop=(m == nquad - 1),
                                        skip_group_check=True,
                                    )
                            if kt >= LAG:
                                consume(kt - LAG)
                        for kt in range(max(0, nkt - LAG), nkt):
                            consume(kt)
                        qs = slice(qc * 512, (qc + 1) * 512)
                        rec = smallp.tile([1, 512], f32, tag="rec")
                        nc.vector.reciprocal_approx_fast(rec[:], den[:])
                        bc = smallp.tile([128, 512], f32, tag="bc")
                        nc.gpsimd.partition_broadcast(bc[:], rec[:])
                        nc.vector.tensor_mul(OT[h][:, qs], oacc[:], bc[:])

                # software pipeline over (pair, chunk)
                slots = [(p, qc) for p in range(2) for qc in range(QC)]
                wqks = {0: wqk0, 1: wqk1}
                prev = None
                for p, qc in slots:
                    proj(p, qc, *wqks[p])
                    if prev is not None:
                        attn(*prev)
                    prev = (p, qc)
                attn(*prev)

            # ---- output projection tail (weights already resident) ------
            for et in range(ET):
                for qc in range(QC):
                    qs = slice(qc * 512, (qc + 1) * 512)
                    facc = psum.tile(
                        [128, 512], f32, tag=f"t{(et * QC + qc) % 4}",
                        name="facc",
                    )
                    for hh in range(H_CORE):
                        nc.tensor.matmul(
                            facc[:],
                            wo_t[et * H_CORE + hh][:],
                            OT[hh][:, qs],
                            start=(hh == 0),
                            stop=(hh == H_CORE - 1),
                        )
                    st = stg.tile([128, 512], bf16, tag="stg")
                    if (et * QC + qc) % 3 == 2:
                        nc.vector.tensor_copy(st[:], facc[:])
                    else:
                        nc.scalar.copy(st[:], facc[:])
                    nc.sync.dma_start(
                        outT_d[et * 128 : (et + 1) * 128, qs], st[:]
                    )

    return nc


_NC = None


def _get_nc():
    global _NC
    if _NC is None:
        _NC = _build()
        _NC.compile()
    return _NC


def _rope_perm_rows():
    """Row permutation applied to wq/wk for one core's 4 heads.

    Per head-pair p: [hA even dims, hB even dims, hA odd dims, hB odd dims]
    so the device sees even/odd deinterleaved, pair-stacked projections.
    Returns indices into the local (4*DK,) head-row block.
    """
    idx = []
    for p in range(2):
        ha, hb = 2 * p, 2 * p + 1
        idx.extend(ha * DK + np.arange(0, DK, 2))
        idx.extend(hb * DK + np.arange(0, DK, 2))
        idx.extend(ha * DK + np.arange(1, DK, 2))
        idx.extend(hb * DK + np.arange(1, DK, 2))
    return np.asarray(idx)


def _host_tables(positions):
    """cos/sin tables (64, S) float32, matching the fp32 reference math."""
    dim_idx = np.arange(0, DK, 2, dtype=np.float32)
    freqs = np.float32(THETA) ** (dim_idx / np.float32(DK))
    angles = positions.astype(np.float32)[:, None] / freqs[None, :]  # (S, 64)
    return (
        np.ascontiguousarray(np.cos(angles).T.astype(np.float32)),
        np.ascontiguousarray(np.sin(angles).T.astype(np.float32)),
    )


def _make_in_maps(inputs):
    x = np.asarray(inputs["x"], dtype=np.float32)
    wq = np.asarray(inputs["wq"], dtype=np.float32)
    wk = np.asarray(inputs["wk"], dtype=np.float32)
    wv = np.asarray(inputs["wv"], dtype=np.float32)
    wo = np.asarray(inputs["wo"], dtype=np.float32)
    token_positions = np.asarray(inputs["token_positions"])

    perm = _rope_perm_rows()
    p_idx = np.arange(128)[:, None]
    f_idx = np.arange(128)[None, :]
    mz = np.zeros((128, 512), dtype=np.float32)
    mz[:, 384:512] = p_idx <= f_idx
    mz = mz.astype(ml_dtypes.bfloat16)

    in_maps = []
    for c in range(N_CORES):
        b = c // 4
        g = c % 4
        rows = slice(g * DL, (g + 1) * DL)
        cosT, sinT = _host_tables(token_positions[b])
        in_maps.append(
            {
                "xT": np.ascontiguousarray(x[b].T).astype(ml_dtypes.bfloat16),
                "wqT": np.ascontiguousarray(wq[rows][perm].T).astype(
                    ml_dtypes.bfloat16
                ),
                "wkT": np.ascontiguousarray(wk[rows][perm].T).astype(
                    ml_dtypes.bfloat16
                ),
                "wvT": np.ascontiguousarray(wv[rows].T).astype(
                    ml_dtypes.bfloat16
                ),
                "woT": np.ascontiguousarray(wo[:, rows].T).astype(
                    ml_dtypes.bfloat16
                ),
                "cosT": cosT.astype(ml_dtypes.bfloat16),
                "sinT": sinT.astype(ml_dtypes.bfloat16),
                "tri": mz,
            }
        )
    return in_maps


def kernel(x, wq, wk, wv, wo, token_positions):
    nc = _get_nc()
    in_maps = _make_in_maps(
        {
            "x": x,
            "wq": wq,
            "wk": wk,
            "wv": wv,
            "wo": wo,
            "token_positions": token_positions,
        }
    )
    res = run_bass_kernel_spmd(nc, in_maps, list(range(N_CORES)))

    out = np.zeros((B, S, D), dtype=np.float32)
    for c in range(N_CORES):
        out[c // 4] += res.results[c]["outT"].astype(np.float32).T
    return out


# revision 12
# speedup vs baseline: 1.6162x; 1.0462x over previous
"""Multi-head self-attention with RoPE on 8 Trainium2 NeuronCores.

Problem: B=2, S=2048, D_MODEL=2048, 16 heads x d_k=128, causal, RoPE on Q/K.

Sharding (hardcoded): core c -> batch b=c//4, head group g=c%4 (heads 4g..4g+3).
Data parallel on batch, tensor parallel on heads; q/k/v projections column-
sharded, output projection row-sharded with the partial sums reduced on host.

Device kernel (identical program on all 8 cores, different data). All matmul
operands are bf16 (PSUM accumulation stays fp32; end-to-end rel err ~5e-3,
well inside the 2e-2 gate), which halves DMA traffic vs fp32 and enables the
PE fast-weight-load path that fp32r weights cannot use. Layouts avoid all
on-chip transposes:
    QT/KT: (d_k, S) per head  - projection with weight-subtile stationary
    V:     (S, d_k*4) natural - projection with x-subtile stationary
    S^T = KT_tile.T @ QT -> (k, q); softmax sums over partitions via a
    ones-vector matmul; AV: V_tile.T @ P~T -> (d, q); output projection
    woT_tile.T @ OT -> partial out^T (e, q).

Schedule highlights:
  - x^T is loaded into SBUF once (bf16, 8.4MB) and stays resident; the
    V-pass runs et-outer over chunk pairs so compute starts ~2us into the
    x DMA stream instead of waiting for all of it.
  - Attention for chunk qc is emitted after the projection of chunk qc+1,
    so the RoPE drain (DVE) of a chunk overlaps the next chunk's
    projection matmuls and the PE never waits on the vector engine.
  - Causal diagonal blocks are trimmed to their live column range
    (512/384/256/128 wide); only the leading 128 columns of each are
    triangular, masked with one shared (128,128) lower-triangle multiply.
  - softmax normalization uses reciprocal_approx_fast (the exact DVE
    reciprocal costs 3.3us per call on a (1,512) operand).
  - RoPE: even/odd d_k interleave is pre-permuted into the wq/wk rows on
    host; rows 0:64 = even dims, 64:128 = odd dims per head.
"""

import sys

sys.path.insert(0, "/opt/trn_rl_repo")

import math
from contextlib import ExitStack

import ml_dtypes
import numpy as np

import concourse.bass as bass
import concourse.mybir as mybir
import concourse.tile as tile
from concourse import bacc
from concourse.bass_utils import run_bass_kernel_spmd

f32 = mybir.dt.float32
bf16 = mybir.dt.bfloat16

B = 2
S = 2048
D = 2048
H = 16
DK = 128
H_CORE = 4  # heads per core
DL = H_CORE * DK  # local feature dim 512
ET = D // 128  # 16 e-tiles (contraction over d_model)
QC = S // 512  # 4 q-chunks
THETA = 10000.0
SCALE = 1.0 / math.sqrt(DK)

N_CORES = 8


def _build():
    nc = bacc.Bacc("TRN2", target_bir_lowering=False, debug=False)

    xT_d = nc.dram_tensor("xT", [D, S], bf16, kind="ExternalInput")
    wqT_d = nc.dram_tensor("wqT", [D, DL], bf16, kind="ExternalInput")
    wkT_d = nc.dram_tensor("wkT", [D, DL], bf16, kind="ExternalInput")
    wvT_d = nc.dram_tensor("wvT", [D, DL], bf16, kind="ExternalInput")
    woT_d = nc.dram_tensor("woT", [DL, D], bf16, kind="ExternalInput")
    cosT_d = nc.dram_tensor("cosT", [64, S], bf16, kind="ExternalInput")
    sinT_d = nc.dram_tensor("sinT", [64, S], bf16, kind="ExternalInput")
    tri_d = nc.dram_tensor("tri", [128, 512], bf16, kind="ExternalInput")
    outT_d = nc.dram_tensor("outT", [D, S], bf16, kind="ExternalOutput")

    Exp = mybir.ActivationFunctionType.Exp

    with tile.TileContext(nc) as tc:
      with tc.tile_pool(name="const", bufs=1) as const, \
           tc.tile_pool(name="xpool", bufs=1) as xpool, \
           tc.tile_pool(name="persist", bufs=1) as persist, \
           tc.tile_pool(name="wqp", bufs=24) as wqp, \
           tc.tile_pool(name="wkp", bufs=24) as wkp, \
           tc.tile_pool(name="ropet", bufs=1) as ropet, \
           tc.tile_pool(name="psum", bufs=1, space="PSUM") as psum:

        xs = [xpool.tile([128, S], bf16, tag=f"x{et}", name=f"x{et}")
              for et in range(ET)]
        V = [persist.tile([128, DL], bf16, tag=f"v{st}", name=f"v{st}")
             for st in range(ET)]
        OT = [persist.tile([DK, S], bf16, tag=f"ot{h}", name=f"ot{h}")
              for h in range(H_CORE)]

        def load_wqk(p):
            wq_sb, wk_sb = [], []
            for et in range(ET):
                wqt = wqp.tile([128, 256], bf16, tag="wq", name=f"wq{p}")
                nc.sync.dma_start(
                    wqt[:],
                    wqT_d[et * 128 : (et + 1) * 128, p * 256 : (p + 1) * 256],
                )
                wq_sb.append(wqt)
                wkt = wkp.tile([128, 256], bf16, tag="wk", name=f"wk{p}")
                nc.sync.dma_start(
                    wkt[:],
                    wkT_d[et * 128 : (et + 1) * 128, p * 256 : (p + 1) * 256],
                )
                wk_sb.append(wkt)
            return wq_sb, wk_sb

        # ---- V pass: et-outer over chunk pairs so the PE starts after the
        # first x/wv tiles land rather than after the full 10.5MB stream.
        with tc.tile_pool(name="wvp", bufs=16) as wvp:
            wv_sb = []
            for et in range(ET):
                wt = wvp.tile([128, DL], bf16, tag="wv", name="wv")
                nc.sync.dma_start(wt[:], wvT_d[et * 128 : (et + 1) * 128, :])
                wv_sb.append(wt)
                nc.sync.dma_start(
                    xs[et][:, 0:1024], xT_d[et * 128 : (et + 1) * 128, 0:1024]
                )
            for et in range(ET):
                nc.sync.dma_start(
                    xs[et][:, 1024:2048],
                    xT_d[et * 128 : (et + 1) * 128, 1024:2048],
                )
            for qcp in range(2):  # chunk pairs (0,1) then (2,3)
                vacc = [
                    psum.tile([128, DL], f32, tag=f"t{i}", name=f"vacc{i}")
                    for i in range(8)
                ]
                for et in range(ET):
                    for half in range(2):
                        qc = 2 * qcp + half
                        for sl in range(4):
                            nc.tensor.matmul(
                                vacc[4 * half + sl][:],
                                xs[et][:, qc * 512 + sl * 128
                                       : qc * 512 + (sl + 1) * 128],
                                wv_sb[et][:],
                                start=(et == 0),
                                stop=(et == ET - 1),
                            )
                for half in range(2):
                    for sl in range(4):
                        nc.scalar.copy(
                            V[(2 * qcp + half) * 4 + sl][:],
                            vacc[4 * half + sl][:],
                        )
            wqk0 = load_wqk(0)

        # ---- constants + prefetches (land during the V pass) ------------
        cos2 = const.tile([128, S], bf16, tag="cos2")
        sin2 = const.tile([128, S], bf16, tag="sin2")
        nc.sync.dma_start(cos2[0:64, :], cosT_d[:, :])
        nc.sync.dma_start(cos2[64:128, :], cosT_d[:, :])
        nc.sync.dma_start(sin2[0:64, :], sinT_d[:, :])
        nc.sync.dma_start(sin2[64:128, :], sinT_d[:, :])
        mz = const.tile([128, 512], bf16, tag="mz")
        nc.sync.dma_start(mz[:], tri_d[:, :])
        ones_f = const.tile([128, 1], f32, tag="ones_f")
        nc.vector.memset(ones_f[:], 1.0)
        ones = const.tile([128, 1], bf16, tag="ones")
        nc.vector.tensor_copy(ones[:], ones_f[:])
        wqk1 = load_wqk(1)
        with tc.tile_pool(name="wop", bufs=64) as wop, \
             tc.tile_pool(name="stg", bufs=6) as stg:
            wo_t = []
            for et in range(ET):
                for hh in range(H_CORE):
                    wt = wop.tile([128, 128], bf16, tag="wo", name="wo")
                    nc.sync.dma_start(
                        wt[:],
                        woT_d[hh * 128 : (hh + 1) * 128,
                              et * 128 : (et + 1) * 128],
                    )
                    wo_t.append(wt)

            def rope(dst, ev, od, qs):
                """ev/od: PSUM accumulators (128,512), rows [hA;hB]."""
                c = cos2[:, qs]
                sn = sin2[:, qs]
                m1 = ropet.tile([128, 512], bf16, tag="m1")
                m2 = ropet.tile([128, 512], bf16, tag="m2")
                n1 = ropet.tile([128, 512], bf16, tag="n1")
                n2 = ropet.tile([128, 512], bf16, tag="n2")
                nc.vector.tensor_mul(m1[:], ev[:], c)
                nc.vector.tensor_mul(m2[:], od[:], sn)
                nc.vector.tensor_mul(n1[:], ev[:], sn)
                nc.vector.tensor_mul(n2[:], od[:], c)
                nc.vector.tensor_sub(dst[0][0:64, qs], m1[0:64, :],
                                     m2[0:64, :])
                nc.vector.tensor_sub(dst[1][0:64, qs], m1[64:128, :],
                                     m2[64:128, :])
                nc.vector.tensor_add(dst[0][64:128, qs], n1[0:64, :],
                                     n2[0:64, :])
                nc.vector.tensor_add(dst[1][64:128, qs], n1[64:128, :],
                                     n2[64:128, :])

            # ---- projection / rope / attention pipeline ----------------
            # Emission (= per-engine priority) order lags each chunk's
            # attention one slot behind its projection:
            #   proj(0) rope(0) proj(1) attn(0) rope(1) proj(2) attn(1) ...
            # so rope(qc) on DVE overlaps proj(qc+1) on the PE, and the
            # proj accumulators (t0-t3) are freed by rope's 4 leading muls.
            with tc.tile_pool(name="qk", bufs=2) as qkp, \
                 tc.tile_pool(name="ptp", bufs=8) as ptp, \
                 tc.tile_pool(name="paccp", bufs=2) as paccp, \
                 tc.tile_pool(name="smallp", bufs=2) as smallp:

                QTs, KTs = {}, {}

                def proj(p, qc, wq_sb, wk_sb):
                    if qc == 0:
                        QTs[p] = [qkp.tile([DK, S], bf16, tag=f"qt{i}",
                                           name=f"qt{p}_{i}")
                                  for i in range(2)]
                        KTs[p] = [qkp.tile([DK, S], bf16, tag=f"kt{i}",
                                           name=f"kt{p}_{i}")
                                  for i in range(2)]
                    qs = slice(qc * 512, (qc + 1) * 512)
                    qe = psum.tile([128, 512], f32, tag="t0", name="qe")
                    qo = psum.tile([128, 512], f32, tag="t1", name="qo")
                    ke = psum.tile([128, 512], f32, tag="t2", name="ke")
                    ko = psum.tile([128, 512], f32, tag="t3", name="ko")
                    for et in range(ET):
                        xt = xs[et]
                        nc.tensor.matmul(
                            qe[:], wq_sb[et][:, 0:128], xt[:, qs],
                            start=(et == 0), stop=(et == ET - 1),
                        )
                        nc.tensor.matmul(
                            qo[:], wq_sb[et][:, 128:256], xt[:, qs],
                            start=(et == 0), stop=(et == ET - 1),
                        )
                        nc.tensor.matmul(
                            ke[:], wk_sb[et][:, 0:128], xt[:, qs],
                            start=(et == 0), stop=(et == ET - 1),
                        )
                        nc.tensor.matmul(
                            ko[:], wk_sb[et][:, 128:256], xt[:, qs],
                            start=(et == 0), stop=(et == ET - 1),
                        )
                    rope(QTs[p], qe, qo, qs)
                    rope(KTs[p], ke, ko, qs)

                def attn(p, qc):
                    QT, KT = QTs[p], KTs[p]
                    ha = 2 * p
                    for hi, h in enumerate((ha, ha + 1)):
                        nkt = 4 * qc + 4
                        nquad = nkt // 4
                        LAG = 2  # scores run LAG blocks ahead of AV
                        den = psum.tile([1, 512], f32, tag="t6", name="den")
                        oacc = psum.tile([128, 512], f32, tag="t7",
                                         name="oacc")
                        pts = {}
                        pacc = [None] * nquad

                        def consume(kt):
                            j = kt - 4 * qc
                            o = 128 * j if j >= 0 else 0
                            cs = slice(o, 512)
                            pt = pts.pop(kt)
                            nc.tensor.matmul(
                                oacc[:, cs],
                                V[kt][:, h * 128 : (h + 1) * 128],
                                pt[:, cs],
                                start=(kt == 0),
                                stop=(kt == nkt - 1),
                                skip_group_check=True,
                            )

                        for kt in range(nkt):
                            j = kt - 4 * qc
                            o = 128 * j if j >= 0 else 0
                            cs = slice(o, 512)
                            sps = psum.tile(
                                [128, 512], f32,
                                tag=("t4" if kt % 2 == 0 else "t5"),
                                name="sps",
                            )
                            nc.tensor.matmul(
                                sps[:, cs],
                                KT[hi][:, kt * 128 : (kt + 1) * 128],
                                QT[hi][:, qc * 512 + o : (qc + 1) * 512],
                                start=True,
                                stop=True,
                            )
                            pt = ptp.tile([128, 512], bf16, tag="pt")
                            nc.scalar.activation(
                                pt[:, cs], sps[:, cs], Exp, scale=SCALE
                            )
                            if j >= 0:
                                # only cols [o, o+128) of a diagonal block
                                # are triangular; the rest is causal-valid
                                nc.vector.tensor_mul(
                                    pt[:, o : o + 128],
                                    pt[:, o : o + 128],
                                    mz[:, 384:512],
                                )
                            pts[kt] = pt
                            # den pre-accumulation: 3 DVE adds per quad of
                            # k-tiles replace 3 of 4 PE den streams. The
                            # diagonal quad's tiles are only valid on
                            # [128*j, 512), so its adds are range-limited
                            # (cols below that were never written).
                            m, r = divmod(kt, 4)
                            diag = m == qc
                            if r == 1:
                                pa = paccp.tile([128, 512], bf16,
                                                tag="pacc")
                                if diag:
                                    nc.vector.tensor_copy(
                                        pa[:, 0:128], pts[kt - 1][:, 0:128]
                                    )
                                    nc.vector.tensor_add(
                                        pa[:, 128:512],
                                        pts[kt - 1][:, 128:512],
                                        pt[:, 128:512],
                                    )
                                else:
                                    nc.vector.tensor_add(
                                        pa[:], pts[kt - 1][:], pt[:]
                                    )
                                pacc[m] = pa
                            elif r > 1:
                                cs2 = slice(o, 512) if diag else slice(
                                    0, 512
                                )
                                nc.vector.tensor_add(
                                    pacc[m][:, cs2],
                                    pacc[m][:, cs2],
                                    pt[:, cs2],
                                )
                                if r == 3:
                                    nc.tensor.matmul(
                                        den[:], ones[:], pacc[m][:],
                                        start=(m == 0),
                                        stop=(m == nquad - 1),
                                        skip_group_check=True,
                                    )
                            if kt >= LAG:
                                consume(kt - LAG)
                        for kt in range(max(0, nkt - LAG), nkt):
                            consume(kt)
                        qs = slice(qc * 512, (qc + 1) * 512)
                        rec = smallp.tile([1, 512], f32, tag="rec")
                        nc.vector.reciprocal_approx_fast(rec[:], den[:])
                        bc = smallp.tile([128, 512], f32, tag="bc")
                        nc.gpsimd.partition_broadcast(bc[:], rec[:])
                        nc.vector.tensor_mul(OT[h][:, qs], oacc[:], bc[:])

                # software pipeline over (pair, chunk)
                slots = [(p, qc) for p in range(2) for qc in range(QC)]
                wqks = {0: wqk0, 1: wqk1}
                prev = None
                for p, qc in slots:
                    proj(p, qc, *wqks[p])
                    if prev is not None:
                        attn(*prev)
                    prev = (p, qc)
                attn(*prev)

            # ---- output projection tail (weights already resident) ------
            # qc-outer: chunks 0-2 are ready long before the last
            # attention chunk finishes, so the PE can start here while
            # attn(1,3)'s softmax tail drains on Act/DVE.
            for idx, (qc, et) in enumerate(
                (qc, et) for qc in range(QC) for et in range(ET)
            ):
                    qs = slice(qc * 512, (qc + 1) * 512)
                    facc = psum.tile(
                        [128, 512], f32, tag=f"t{idx % 4}",
                        name="facc",
                    )
                    for hh in range(H_CORE):
                        nc.tensor.matmul(
                            facc[:],
                            wo_t[et * H_CORE + hh][:],
                            OT[hh][:, qs],
                            start=(hh == 0),
                            stop=(hh == H_CORE - 1),
                        )
                    st = stg.tile([128, 512], bf16, tag="stg")
                    if idx % 3 == 2:
                        nc.vector.tensor_copy(st[:], facc[:])
                    else:
                        nc.scalar.copy(st[:], facc[:])
                    nc.sync.dma_start(
                        outT_d[et * 128 : (et + 1) * 128, qs], st[:]
                    )

    return nc


_NC = None


def _get_nc():
    global _NC
    if _NC is None:
        _NC = _build()
        _NC.compile()
    return _NC


def _rope_perm_rows():
    """Row permutation applied to wq/wk for one core's 4 heads.

    Per head-pair p: [hA even dims, hB even dims, hA odd dims, hB odd dims]
    so the device sees even/odd deinterleaved, pair-stacked projections.
    Returns indices into the local (4*DK,) head-row block.
    """
    idx = []
    for p in range(2):
        ha, hb = 2 * p, 2 * p + 1
        idx.extend(ha * DK + np.arange(0, DK, 2))
        idx.extend(hb * DK + np.arange(0, DK, 2))
        idx.extend(ha * DK + np.arange(1, DK, 2))
        idx.extend(hb * DK + np.arange(1, DK, 2))
    return np.asarray(idx)


def _host_tables(positions):
    """cos/sin tables (64, S) float32, matching the fp32 reference math."""
    dim_idx = np.arange(0, DK, 2, dtype=np.float32)
    freqs = np.float32(THETA) ** (dim_idx / np.float32(DK))
    angles = positions.astype(np.float32)[:, None] / freqs[None, :]  # (S, 64)
    return (
        np.ascontiguousarray(np.cos(angles).T.astype(np.float32)),
        np.ascontiguousarray(np.sin(angles).T.astype(np.float32)),
    )


def _make_in_maps(inputs):
    x = np.asarray(inputs["x"], dtype=np.float32)
    wq = np.asarray(inputs["wq"], dtype=np.float32)
    wk = np.asarray(inputs["wk"], dtype=np.float32)
    wv = np.asarray(inputs["wv"], dtype=np.float32)
    wo = np.asarray(inputs["wo"], dtype=np.float32)
    token_positions = np.asarray(inputs["token_positions"])

    perm = _rope_perm_rows()
    p_idx = np.arange(128)[:, None]
    f_idx = np.arange(128)[None, :]
    mz = np.zeros((128, 512), dtype=np.float32)
    mz[:, 384:512] = p_idx <= f_idx
    mz = mz.astype(ml_dtypes.bfloat16)

    in_maps = []
    for c in range(N_CORES):
        b = c // 4
        g = c % 4
        rows = slice(g * DL, (g + 1) * DL)
        cosT, sinT = _host_tables(token_positions[b])
        in_maps.append(
            {
                "xT": np.ascontiguousarray(x[b].T).astype(ml_dtypes.bfloat16),
                "wqT": np.ascontiguousarray(wq[rows][perm].T).astype(
                    ml_dtypes.bfloat16
                ),
                "wkT": np.ascontiguousarray(wk[rows][perm].T).astype(
                    ml_dtypes.bfloat16
                ),
                "wvT": np.ascontiguousarray(wv[rows].T).astype(
                    ml_dtypes.bfloat16
                ),
                "woT": np.ascontiguousarray(wo[:, rows].T).astype(
                    ml_dtypes.bfloat16
                ),
                "cosT": cosT.astype(ml_dtypes.bfloat16),
                "sinT": sinT.astype(ml_dtypes.bfloat16),
                "tri": mz,
            }
        )
    return in_maps


def kernel(x, wq, wk, wv, wo, token_positions):
    nc = _get_nc()
    in_maps = _make_in_maps(
        {
            "x": x,
            "wq": wq,
            "wk": wk,
            "wv": wv,
            "wo": wo,
            "token_positions": token_positions,
        }
    )
    res = run_bass_kernel_spmd(nc, in_maps, list(range(N_CORES)))

    out = np.zeros((B, S, D), dtype=np.float32)
    for c in range(N_CORES):
        out[c // 4] += res.results[c]["outT"].astype(np.float32).T
    return out
